# revision 10
# baseline (speedup 1.0000x reference)
"""Distributed Bass kernel for nn_AllLoss: YOLACT-style loss over 8 cores.

Per-core (one image each):
  cls:  -ln(p[pos]).sum()/K/K  +  -ln(1-p[neg]).sum()/3K/K
  loc:  smooth_l1(pr - encode(gt, anchor)).sum()/K
  msk:  BCE(sigmoid(coef@proto), goalmask).mean(hw).sum(k)/K
        = [ sum softplus(z) - sum_k <y_gk, z_k> ] / 16384 / K
        with  sum_k <y_gk, z_k> = <c_agg, G>,  G[b,p] = <y_b, proto_p>,
              c_agg = onehot(gt_idx)^T @ coef_gathered

v4 schedule (from v1-v3 trace analysis):
  - critical path to the first EXP is just: idx DMA -> coef gathers ->
    transposes -> c_agg -> z round 0.  The G-chain (only needed for the
    S2 correction) runs AFTER the z stream, overlapping the Ln tail.
  - z matmuls are row-group packed 2x: proto replicated on partitions
    0-3 and 32-35, two concurrent matmul chains (row groups 0/1).
  - G is col-group packed 4x (four accumulation chains into partitions
    32c..32c+3 of one PSUM tile).
  - sum softplus via ln of products-of-16 of (1+e^z) in bf16;
    rounds 14-15 stop at products-of-8 so the tail Ln starts sooner.
  - DMA submissions cost ~0.6us each on a sequencer: proto_g+masks_g
    ride one combined DMA; proto_z goes on the scalar queue.
  - out = per-partition stats [128,9]; host applies W_S1 to cols 6:9
    and sums all cores' stats.
"""
import sys

sys.path.insert(0, "/opt/trn_rl_repo")
import numpy as np
from concourse import bacc, mybir, tile

# Pin the single activation table we use (Exp + Ln live together here).
_orig_gat = bacc.get_activation_tables


def _gat_one_set(arch):
    t = _orig_gat(arch)
    keep = "natural_log_exp_and_others"
    return {k: (v if k == keep else set()) for k, v in t.items()}


bacc.get_activation_tables = _gat_one_set

N, A, K, B, P, HW = 8, 16368, 200, 20, 4, 128
HW2 = HW * HW  # 16384
KN = 3 * K  # 600
F32 = mybir.dt.float32
BF16 = mybir.dt.bfloat16
I32 = mybir.dt.int32
AF = mybir.ActivationFunctionType
ALU = mybir.AluOpType

# weights fold the final /8 mean over cores
W_POS = -1.0 / (K * K * N)        # stats hold +ln(p)
W_NEG = -1.0 / (KN * K * N)       # stats hold +ln(1-p)
W_LOC = 1.0 / (K * N)
W_S1 = 1.0 / (HW2 * K * N)        # host applies to stats cols 6:9
W_S2 = -W_S1
INV_LN10 = float(1.0 / np.log(10.0))
LN2 = float(np.log(2.0))
# deg-5 fit of ln(m) on [1,2), max abs err ~1e-5
LNC = [0.030449, -0.28382685, 1.11609003, -2.44002976, 3.5140873, -1.93675974]

ZSLOTS = 8
ZROUNDS = HW // ZSLOTS  # 16
PF_COLS = P * HW2 // 128   # 512
MG_COLS = B * HW2 // 128   # 2560
CB_COLS = PF_COLS + MG_COLS  # 3072


def build_kernel():
    nc = bacc.Bacc(None, target_bir_lowering=False, debug=False)

    big = nc.declare_dram_parameter("big", [A, 13], F32, isOutput=False)
    cls = nc.declare_dram_parameter("cls", [A, 1], F32, isOutput=False)
    proto_z = nc.declare_dram_parameter("proto_z", [P, HW2], BF16,
                                        isOutput=False)
    # combo_b = proto_g [128, 512] ++ masks_g [128, 2560], both bf16
    combo_b = nc.declare_dram_parameter("combo_b", [128, CB_COLS], BF16,
                                        isOutput=False)
    gtb = nc.declare_dram_parameter("gtb", [B, 4], F32, isOutput=False)
    # packed indices [128, 9]: c0 pos[:128], c1 pos[128:]+pad, c2 gt[:128],
    # c3 gt[128:]+pad, c4:9 neg (120 rows per col)
    idx = nc.declare_dram_parameter("idx", [128, 9], I32, isOutput=False)
    ident_d = nc.declare_dram_parameter("ident", [128, 128], F32,
                                        isOutput=False)
    iota_d = nc.declare_dram_parameter("iota", [128, B], F32, isOutput=False)
    out = nc.declare_dram_parameter("out", [128, 9], F32, isOutput=True)

    with tile.TileContext(nc) as tc:
        with tc.tile_pool(name="sb", bufs=1) as sb:
            # ---------------- DMAs, split across sync + scalar queues -----
            idxt = sb.tile([128, 9], I32)
            nc.sync.dma_start(out=idxt[:], in_=idx[:, :])
            comboB = sb.tile([128, CB_COLS], BF16)
            nc.sync.dma_start(out=comboB[:], in_=combo_b[:, :])
            ident = sb.tile([128, 128], F32)
            nc.sync.dma_start(out=ident[:], in_=ident_d[:, :])
            iota_f = sb.tile([128, B], F32)
            nc.sync.dma_start(out=iota_f[:], in_=iota_d[:, :])
            # proto for the z stream, replicated on partitions 0-3 / 32-35
            pz = sb.tile([36, HW2], BF16)
            nc.scalar.dma_start(out=pz[0:4, :], in_=proto_z[:, :])
            nc.scalar.dma_start(out=pz[32:36, :], in_=proto_z[:, :])
            protoGb = comboB[:, 0:PF_COLS]
            masksGb = comboB[:, PF_COLS:CB_COLS]
            posi1 = idxt[:, 0:1]
            posi2 = idxt[0:72, 1:2]
            gti1 = idxt[:, 2:3]
            gti2 = idxt[0:72, 3:4]

            stats = sb.tile([128, 9], F32)
            nc.vector.memset(stats[:], 0.0)
            # dummy exp so the act table loads during staging
            warm = sb.tile([1, 2], F32)
            nc.vector.memset(warm[:], 0.0)
            nc.scalar.activation(warm[0:1, 1:2], warm[0:1, 0:1], AF.Exp)

            # one-hot H for c_agg (DVE, early, cheap)
            gidx1 = sb.tile([128, 1], F32)
            gidx2 = sb.tile([72, 1], F32)
            nc.vector.tensor_copy(out=gidx1[:], in_=gti1)
            nc.vector.tensor_copy(out=gidx2[:], in_=gti2)
            H1 = sb.tile([128, B], F32)
            H2 = sb.tile([72, B], F32)
            nc.vector.tensor_scalar(out=H1[:], in0=iota_f[:],
                                    scalar1=gidx1[:, :1], scalar2=None,
                                    op0=ALU.is_equal)
            nc.vector.tensor_scalar(out=H2[:], in0=iota_f[0:72, :],
                                    scalar1=gidx2[:, :1], scalar2=None,
                                    op0=ALU.is_equal)

            # -------- gathers (gpsimd FIFO; coef path first, rest later) ---
            bigg1 = sb.tile([128, 13], F32)
            bigg2 = sb.tile([72, 13], F32)
            nc.gpsimd.indirect_dma_start(
                out=bigg1[:], out_offset=None, in_=big[:, :],
                in_offset=bacc.bass.IndirectOffsetOnAxis(ap=posi1, axis=0))
            nc.gpsimd.indirect_dma_start(
                out=bigg2[:], out_offset=None, in_=big[:, :],
                in_offset=bacc.bass.IndirectOffsetOnAxis(ap=posi2, axis=0))
            # NOTE: the remaining 7 gathers are emitted AFTER the z-loop:
            # consumers wait on the shared swdge semaphore count as of
            # their program position, so any gather enqueued before the
            # transposes would gate the whole z stream.
            gtg1 = sb.tile([128, 4], F32)
            gtg2 = sb.tile([72, 4], F32)
            negp = sb.tile([120, 5], F32)

            # ---------- PE startup: coef transposes + c_agg only ----------
            coefT = sb.tile([36, K], BF16)
            caggSb = sb.tile([128, B], F32)
            with tc.tile_pool(name="psA", bufs=1, space="PSUM") as psA:
                ctps = psA.tile([36, 256], F32)
                for base in (0, 32):
                    # coef^T via plain matmul against identity (transpose-
                    # mode matmuls cannot target PSUM partition 32)
                    nc.tensor.matmul(out=ctps[base:base + 4, 0:128],
                                     lhsT=bigg1[:, 4:8], rhs=ident[:],
                                     start=True, stop=True,
                                     tile_position=(0, base))
                    nc.tensor.matmul(out=ctps[base:base + 4, 128:200],
                                     lhsT=bigg2[:, 4:8],
                                     rhs=ident[0:72, 0:72],
                                     start=True, stop=True,
                                     tile_position=(0, base))
                    nc.vector.tensor_copy(out=coefT[base:base + 4, :],
                                          in_=ctps[base:base + 4, 0:200])
                # c_agg^T[p, b] = sum_k coef[k, p] * onehot[k, b],
                # replicated into 4 col-groups for the post-bulk S2
                caggPs = psA.tile([128, B], F32)
                for c in range(4):
                    nc.tensor.matmul(out=caggPs[32 * c:32 * c + 4, :],
                                     lhsT=bigg1[:, 4:8], rhs=H1[:],
                                     start=True, stop=False,
                                     tile_position=(0, 32 * c))
                    nc.tensor.matmul(out=caggPs[32 * c:32 * c + 4, :],
                                     lhsT=bigg2[:, 4:8], rhs=H2[:],
                                     start=False, stop=True,
                                     tile_position=(0, 32 * c))
                    nc.vector.tensor_copy(out=caggSb[32 * c:32 * c + 4, :],
                                          in_=caggPs[32 * c:32 * c + 4, :])

            # ---------------- bulk: z matmuls + exp/products/ln -----------
            # sum softplus(z) = ln prod (1+e^z); products-of-16 in bf16
            # (rounds 14-15 stop at products-of-8 to shorten the tail)
            scr = sb.tile([128, 4 * K], F32)
            lbufA = sb.tile([128, 4, K], BF16)
            lbufB = sb.tile([128, 5, K], BF16)
            sp_instrs = []
            with tc.tile_pool(name="psZ", bufs=2, space="PSUM") as psZ, \
                 tc.tile_pool(name="sb2", bufs=4) as sb2:
                t8_prev = None
                for r in range(ZROUNDS):
                    zp = psZ.tile([128, ZSLOTS, 256], F32, tag="zp", name="zp")
                    for s in range(ZSLOTS // 2):
                        t = r * ZSLOTS + s
                        nc.tensor.matmul(
                            out=zp[:, s, 0:K],
                            lhsT=pz[0:4, t * 128:(t + 1) * 128],
                            rhs=coefT[0:4, :], start=True, stop=True)
                        t2 = t + 4
                        nc.tensor.matmul(
                            out=zp[:, s + 4, 0:K],
                            lhsT=pz[32:36, t2 * 128:(t2 + 1) * 128],
                            rhs=coefT[32:36, :], start=True, stop=True)
                    et = sb2.tile([128, ZSLOTS * K], BF16, tag="et", name="et")
                    sp = nc.scalar.activation(et[:], zp[:, :, 0:K], AF.Exp)
                    sp_instrs.append(sp)
                    # w = 1 + e^z ; t8 = products-of-2
                    w = sb2.tile([128, ZSLOTS * K], BF16, tag="w", name="w")
                    nc.vector.tensor_scalar_add(w[:], et[:], 1.0)
                    t8 = sb2.tile([128, 4 * K], BF16, tag="t8", name="t8")
                    nc.vector.tensor_tensor(out=t8[:], in0=w[:, 0:4 * K],
                                            in1=w[:, 4 * K:8 * K],
                                            op=ALU.mult)
                    if r % 2 == 0:
                        t8_prev = t8
                    else:
                        v4 = sb2.tile([128, 4 * K], BF16, tag="v4", name="v4")
                        nc.vector.tensor_tensor(out=v4[:], in0=t8_prev[:],
                                                in1=t8[:], op=ALU.mult)
                        if r < 14:
                            v8 = sb2.tile([128, 2 * K], BF16, tag="v8",
                                          name="v8")
                            nc.vector.tensor_tensor(
                                out=v8[:], in0=v4[:, 0:2 * K],
                                in1=v4[:, 2 * K:4 * K], op=ALU.mult)
                            if r < 8:
                                lb, q = lbufA, (r // 2) % 4
                            else:
                                lb, q = lbufB, (r - 8) // 2
                            nc.vector.tensor_tensor(out=lb[:, q, :],
                                                    in0=v8[:, 0:K],
                                                    in1=v8[:, K:2 * K],
                                                    op=ALU.mult)
                        else:
                            # rounds 14-15: products-of-8 into 2 slots
                            nc.vector.tensor_tensor(
                                out=lbufB[:, 3:5, :], in0=v4[:, 0:2 * K],
                                in1=v4[:, 2 * K:4 * K], op=ALU.mult)
                    if r == 10:
                        # rounds 0-7 products are certainly done by now:
                        # no ACT-FIFO stall on the DVE chain
                        nc.scalar.activation(scr[:], lbufA[:], AF.Ln,
                                             accum_out=stats[:, 6:7])
                nc.scalar.activation(scr[:, 0:3 * K], lbufB[:, 0:3, :], AF.Ln,
                                     accum_out=stats[:, 7:8])
                nc.scalar.activation(scr[:, 0:2 * K], lbufB[:, 3:5, :], AF.Ln,
                                     accum_out=stats[:, 8:9])

                # deferred gathers (run on gpsimd during the bulk phase)
                nc.gpsimd.indirect_dma_start(
                    out=gtg1[:], out_offset=None, in_=gtb[:, :],
                    in_offset=bacc.bass.IndirectOffsetOnAxis(ap=gti1, axis=0))
                nc.gpsimd.indirect_dma_start(
                    out=gtg2[:], out_offset=None, in_=gtb[:, :],
                    in_offset=bacc.bass.IndirectOffsetOnAxis(ap=gti2, axis=0))
                for j in range(5):
                    nc.gpsimd.indirect_dma_start(
                        out=negp[:, j:j + 1], out_offset=None, in_=cls[:, :],
                        in_offset=bacc.bass.IndirectOffsetOnAxis(
                            ap=idxt[0:120, 4 + j:5 + j], axis=0))

                # ---------------- small losses, all on DVE ----------------
                # lnL layout [128,15]: 0 p1 | 1:3 ahw1 | 3:5 gt1 | 5 p2 |
                #   6:8 ahw2 | 8:10 gt2 | 10:15 (1-pneg)
                lnL = sb.tile([128, 15], F32)
                small = []
                small.append(nc.vector.memset(lnL[:], 1.0))
                small.append(nc.vector.tensor_copy(out=lnL[:, 0:1],
                                                   in_=bigg1[:, 12:13]))
                small.append(nc.vector.tensor_copy(out=lnL[:, 1:3],
                                                   in_=bigg1[:, 10:12]))
                small.append(nc.vector.tensor_copy(out=lnL[:, 3:5],
                                                   in_=gtg1[:, 2:4]))
                small.append(nc.vector.tensor_copy(out=lnL[0:72, 5:6],
                                                   in_=bigg2[:, 12:13]))
                small.append(nc.vector.tensor_copy(out=lnL[0:72, 6:8],
                                                   in_=bigg2[:, 10:12]))
                small.append(nc.vector.tensor_copy(out=lnL[0:72, 8:10],
                                                   in_=gtg2[:, 2:4]))
                small.append(nc.vector.tensor_scalar(
                    out=lnL[0:120, 10:15], in0=negp[:], scalar1=-1.0,
                    scalar2=1.0, op0=ALU.mult, op1=ALU.add))
                # ln via exponent/mantissa split + Horner
                bits = lnL[:].bitcast(I32)
                eI = sb.tile([128, 15], I32)
                small.append(nc.vector.tensor_scalar(
                    out=eI[:], in0=bits, scalar1=23, scalar2=None,
                    op0=ALU.logical_shift_right))
                eF = sb.tile([128, 15], F32)
                small.append(nc.vector.tensor_copy(out=eF[:], in_=eI[:]))
                eT = sb.tile([128, 15], F32)
                small.append(nc.vector.tensor_scalar(
                    out=eT[:], in0=eF[:], scalar1=LN2, scalar2=-127.0 * LN2,
                    op0=ALU.mult, op1=ALU.add))
                mI = sb.tile([128, 15], I32)
                small.append(nc.vector.tensor_scalar(
                    out=mI[:], in0=bits, scalar1=0x007FFFFF, scalar2=0x3F800000,
                    op0=ALU.bitwise_and, op1=ALU.bitwise_or))
                mant = mI[:].bitcast(F32)
                h = sb.tile([128, 15], F32)
                small.append(nc.vector.tensor_scalar(
                    out=h[:], in0=mant, scalar1=LNC[0], scalar2=LNC[1],
                    op0=ALU.mult, op1=ALU.add))
                for c in LNC[2:]:
                    small.append(nc.vector.tensor_tensor(
                        out=h[:], in0=h[:], in1=mant, op=ALU.mult))
                    small.append(nc.vector.tensor_scalar_add(h[:], h[:], float(c)))
                small.append(nc.vector.tensor_tensor(
                    out=h[:], in0=h[:], in1=eT[:], op=ALU.add))
                # h now = ln of every lnL column
                # cls
                small.append(nc.vector.tensor_scalar(
                    out=stats[0:128, 0:1], in0=h[:, 0:1], scalar1=W_POS,
                    scalar2=None, op0=ALU.mult))
                small.append(nc.vector.tensor_scalar(
                    out=stats[0:72, 1:2], in0=h[0:72, 5:6], scalar1=W_POS,
                    scalar2=None, op0=ALU.mult))
                negred = sb.tile([120, 1], F32)
                small.append(nc.vector.tensor_reduce(
                    out=negred[:], in_=h[0:120, 10:15],
                    axis=mybir.AxisListType.X, op=ALU.add))
                small.append(nc.vector.tensor_scalar(
                    out=stats[0:120, 2:3], in0=negred[:], scalar1=W_NEG,
                    scalar2=None, op0=ALU.mult))
                # loc: big cols 0:4 pr, 8:10 ac, 10:12 ahw
                for ci, (bigg, gtg, q, acol, lo) in enumerate(
                        [(bigg1, gtg1, 128, 3, 0), (bigg2, gtg2, 72, 4, 5)]):
                    inv = sb.tile([128, 2], F32, tag=f"inv{ci}", name=f"inv{ci}")
                    small.append(nc.vector.reciprocal(inv[0:q, :], bigg[:, 10:12]))
                    d = sb.tile([128, 4], F32, tag=f"d{ci}", name=f"d{ci}")
                    small.append(nc.vector.tensor_tensor(
                        out=d[0:q, 0:2], in0=gtg[:, 0:2], in1=bigg[:, 8:10],
                        op=ALU.subtract))
                    small.append(nc.vector.tensor_tensor(
                        out=d[0:q, 0:2], in0=d[0:q, 0:2], in1=inv[0:q, :],
                        op=ALU.mult))
                    small.append(nc.vector.tensor_tensor(
                        out=d[0:q, 0:2], in0=bigg[:, 0:2], in1=d[0:q, 0:2],
                        op=ALU.subtract))
                    dln = sb.tile([128, 2], F32, tag=f"dln{ci}", name=f"dln{ci}")
                    small.append(nc.vector.tensor_tensor(
                        out=dln[0:q, :], in0=h[0:q, lo + 3:lo + 5],
                        in1=h[0:q, lo + 1:lo + 3], op=ALU.subtract))
                    small.append(nc.vector.tensor_scalar(
                        out=dln[0:q, :], in0=dln[0:q, :], scalar1=-INV_LN10,
                        scalar2=None, op0=ALU.mult))
                    small.append(nc.vector.tensor_tensor(
                        out=d[0:q, 2:4], in0=bigg[:, 2:4], in1=dln[0:q, :],
                        op=ALU.add))
                    nd = sb.tile([128, 4], F32, tag=f"nd{ci}", name=f"nd{ci}")
                    small.append(nc.vector.tensor_scalar(
                        out=nd[0:q, :], in0=d[0:q, :], scalar1=-1.0,
                        scalar2=None, op0=ALU.mult))
                    ad = sb.tile([128, 4], F32, tag=f"ad{ci}", name=f"ad{ci}")
                    small.append(nc.vector.tensor_tensor(
                        out=ad[0:q, :], in0=d[0:q, :], in1=nd[0:q, :],
                        op=ALU.max))
                    m = sb.tile([128, 4], F32, tag=f"m{ci}", name=f"m{ci}")
                    small.append(nc.vector.tensor_scalar(
                        out=m[0:q, :], in0=ad[0:q, :], scalar1=1.0,
                        scalar2=None, op0=ALU.min))
                    sq = sb.tile([128, 4], F32, tag=f"sq{ci}", name=f"sq{ci}")
                    small.append(nc.vector.tensor_tensor(
                        out=sq[0:q, :], in0=m[0:q, :], in1=m[0:q, :],
                        op=ALU.mult))
                    small.append(nc.vector.tensor_scalar(
                        out=sq[0:q, :], in0=sq[0:q, :], scalar1=0.5,
                        scalar2=None, op0=ALU.mult))
                    small.append(nc.vector.tensor_tensor(
                        out=ad[0:q, :], in0=ad[0:q, :], in1=m[0:q, :],
                        op=ALU.subtract))
                    small.append(nc.vector.tensor_tensor(
                        out=sq[0:q, :], in0=sq[0:q, :], in1=ad[0:q, :],
                        op=ALU.add))
                    red = sb.tile([128, 1], F32, tag=f"red{ci}", name=f"red{ci}")
                    small.append(nc.vector.tensor_reduce(
                        out=red[0:q, :], in_=sq[0:q, :],
                        axis=mybir.AxisListType.X, op=ALU.add))
                    small.append(nc.vector.tensor_scalar(
                        out=stats[0:q, acol:acol + 1], in0=red[0:q, :],
                        scalar1=W_LOC, scalar2=None, op0=ALU.mult))

                # keep the small DVE chain out of the bulk ramp-up
                for si in small:
                    tile.add_dep_helper(si.ins, sp_instrs[2].ins, sync=False,
                                        reason="smalls after bulk start")

            # ---------- post-bulk: G (col-packed 4x) + S2 ----------------
            with tc.tile_pool(name="psB", bufs=1, space="PSUM") as psB:
                Gps = psB.tile([128, B], F32)
                for j in range(HW):
                    c = j % 4
                    nc.tensor.matmul(
                        out=Gps[32 * c:32 * c + 4, :],
                        lhsT=protoGb[:, j * P:(j + 1) * P],
                        rhs=masksGb[:, j * B:(j + 1) * B],
                        start=(j < 4), stop=(j >= HW - 4),
                        tile_position=(0, 32 * c))
                s2 = sb.tile([128, B], F32)
                s2r = sb.tile([128, 1], F32)
                for c in range(4):
                    lo = 32 * c
                    nc.vector.tensor_tensor(out=s2[lo:lo + 4, :],
                                            in0=caggSb[lo:lo + 4, :],
                                            in1=Gps[lo:lo + 4, :],
                                            op=ALU.mult)
                    nc.vector.tensor_reduce(out=s2r[lo:lo + 4, :],
                                            in_=s2[lo:lo + 4, :],
                                            axis=mybir.AxisListType.X,
                                            op=ALU.add)
                    nc.vector.tensor_scalar(out=stats[lo:lo + 4, 5:6],
                                            in0=s2r[lo:lo + 4, :],
                                            scalar1=W_S2, scalar2=None,
                                            op0=ALU.mult)

            nc.sync.dma_start(out=out[:, :], in_=stats[:])

    nc.finalize()
    return nc


def _pack_idx(pos, neg, gt):
    m = np.zeros((128, 9), dtype=np.int32)
    m[:, 0] = pos[0:128]
    m[0:72, 1] = pos[128:200]
    m[:, 2] = gt[0:128]
    m[0:72, 3] = gt[128:200]
    for j in range(5):
        m[0:120, 4 + j] = neg[j * 120:(j + 1) * 120]
    return m


def make_in_maps(map_class, map_box, map_coef, proto, anchor_center, anchor_hw,
                 gt_boxes, gt_masks, pos_idx, neg_idx, gt_idx):
    import ml_dtypes
    bf16 = ml_dtypes.bfloat16
    ident = np.eye(128, dtype=np.float32)
    iota = np.broadcast_to(np.arange(B, dtype=np.float32), (128, B)).copy()
    in_maps = []
    for i in range(N):
        big = np.concatenate(
            [map_box[i], map_coef[i], anchor_center, anchor_hw,
             map_class[i].reshape(A, 1)], axis=1).astype(np.float32)
        proto_g = (proto[i].transpose(2, 1, 0)
                   .reshape(128, PF_COLS).astype(bf16))
        masks_g = (gt_masks[i].transpose(2, 1, 0)
                   .reshape(128, MG_COLS).astype(bf16))
        in_maps.append(dict(
            big=np.ascontiguousarray(big),
            cls=np.ascontiguousarray(map_class[i].reshape(A, 1)),
            proto_z=np.ascontiguousarray(
                proto[i].reshape(P, HW2).astype(bf16)),
            combo_b=np.ascontiguousarray(
                np.concatenate([proto_g, masks_g], axis=1)),
            gtb=np.ascontiguousarray(gt_boxes[i]),
            idx=_pack_idx(pos_idx[i], neg_idx[i], gt_idx[i]),
            ident=ident,
            iota=iota,
        ))
    return in_maps


def core_total(stats_out):
    """Combine one core's [128, 9] stats into its scalar contribution."""
    s = np.asarray(stats_out, dtype=np.float64)
    return float(s[:, 0:6].sum() + W_S1 * s[:, 6:9].sum())


def kernel(**inputs):
    from concourse.bass_utils import run_bass_kernel_spmd
    nc = build_kernel()
    in_maps = make_in_maps(**inputs)
    res = run_bass_kernel_spmd(nc, in_maps, core_ids=list(range(N)))
    return np.float32(sum(core_total(res.results[c]["out"])
                          for c in range(N)))
